# revision 9
# baseline (speedup 1.0000x reference)
"""DiagonalLinear kernel for 8x TRN2 NeuronCores (Bass/Tile).

Math: out[b, i] = sum_j x[b, j] * (weight * mask)[i, j] where
mask[i, lag*N_VARS + i] = 1. So the dense matmul collapses to

    out[b, i] = sum_{lag<P} x[b, lag*N_VARS + i] * wd[i, lag]
    wd[i, lag] = weight[i, lag*N_VARS + i]

i.e. an elementwise multiply-accumulate over P=8 lags — memory-bound on
streaming x (256 MB fp32) once, not a matmul.

Sharding: each of the 8 cores owns a contiguous slice of NV=256 variables
(fully independent given the diagonal mask). Per-core device layout puts
variables on SBUF partitions and batch on the free dim, so the per-lag
multiply is a per-partition-scalar op: lag 0 runs on ScalarE (activation
Copy with scale AP), lags 1..7 run as fused scalar_tensor_tensor
(acc = x*wd + acc) on VectorE. Both engines stay under the ~358 GB/s/core
DMA floor (~36 MB traffic per core), so the kernel is DMA-bound.

Host side: extract the weight diagonal (pure gather), transpose x so each
core's shard is (P*NV, BATCH) contiguous, gather per-core outputs (NV,
BATCH) and transpose back to (BATCH, N_VARS).
"""

import os

import numpy as np

import concourse.bass as bass
import concourse.mybir as mybir
from concourse.bass_utils import run_bass_kernel_spmd
from concourse.tile import TileContext

N_VARS = 2048
P = 8
BATCH = 4096
N_CORES = 8
NV = N_VARS // N_CORES  # 256 variables per core
VT = NV // 128  # 2 partition tiles per core
BB = 1024  # batch tile width (free dim)
NB = BATCH // BB
LAG_GROUP = 2  # lags per load DMA (2 -> 1 MB transfers)

_nc_cache = None
LAST_EXEC_TIME_NS = None


def _split_multi_waits(nc):
    """Walrus in this toolchain accepts at most one sync-wait per
    instruction; hoist extras onto same-engine NoOps placed just before.
    Order-preserving and conservative: the engine stalls at the NoOp on the
    same condition it would have waited on at the instruction itself."""
    for fn in nc.m.functions:
        for blk in fn.blocks:
            out = []
            for ins in blk.instructions:
                si = ins.sync_info
                if si is not None and si.on_wait is not None and len(si.on_wait) > 1:
                    waits = list(si.on_wait)
                    for k, w in enumerate(waits[:-1]):
                        out.append(
                            mybir.InstNoOp(
                                name=f"{ins.name}_hw{k}",
                                engine=ins.engine,
                                ins=[],
                                outs=[],
                                sync_info=mybir.SyncInfo(on_wait=[w], on_update=[]),
                            )
                        )
                    ins.sync_info = mybir.SyncInfo(
                        on_wait=[waits[-1]], on_update=si.on_update
                    )
                out.append(ins)
            blk.instructions[:] = out


def _build_nc():
    nc = bass.Bass()
    xt = nc.dram_tensor("xt", [P * NV, BATCH], mybir.dt.float32, kind="ExternalInput")
    wds = nc.dram_tensor("wds", [128, VT * P], mybir.dt.float32, kind="ExternalInput")
    out = nc.dram_tensor("out_t", [NV, BATCH], mybir.dt.float32, kind="ExternalOutput")
    # view rows as (lag, v): row = lag*NV + v  ->  [v, lag, b]
    xt_v = xt.rearrange("(l v) b -> v l b", l=P)

    n_chunks = VT * NB
    with TileContext(nc) as tc:
        with (
            tc.tile_pool(name="w", bufs=1) as wpool,
            tc.tile_pool(name="x", bufs=3) as xpool,
            tc.tile_pool(name="acc", bufs=2) as apool,
        ):
            wtile = wpool.tile([128, VT * P], mybir.dt.float32)
            # ACT ring: keeps the SP ring free so the first x load issues
            # immediately
            nc.scalar.dma_start(out=wtile[:, :], in_=wds[:, :])
            for ci, (vt, bb) in enumerate(
                (vt, bb) for vt in range(VT) for bb in range(NB)
            ):
                # the last chunk loads per-lag so its accumulation chain
                # streams with the loads instead of waiting for all 8 lags
                lg = 1 if ci == n_chunks - 1 else LAG_GROUP
                t = xpool.tile([128, P, BB], mybir.dt.float32, tag="xload")
                for l0 in range(0, P, lg):
                    nc.sync.dma_start(
                        out=t[:, l0 : l0 + lg, :],
                        in_=xt_v[
                            vt * 128 : (vt + 1) * 128,
                            l0 : l0 + lg,
                            bb * BB : (bb + 1) * BB,
                        ],
                    )
                acc = apool.tile([128, BB], mybir.dt.float32, tag="acc")
                # acc = wd[:, lag0] * x_lag0  (per-partition scalar, 2x fp32)
                nc.vector.tensor_scalar_mul(
                    out=acc[:, :],
                    in0=t[:, 0, :],
                    scalar1=wtile[:, vt * P : vt * P + 1],
                )
                for lag in range(1, P):
                    # acc = (x_lag * wd[:, lag]) + acc  (fused on VectorE)
                    nc.vector.scalar_tensor_tensor(
                        out=acc[:, :],
                        in0=t[:, lag, :],
                        scalar=wtile[:, vt * P + lag : vt * P + lag + 1],
                        in1=acc[:, :],
                        op0=mybir.AluOpType.mult,
                        op1=mybir.AluOpType.add,
                    )
                # store on the ACT HWDGE ring so a hoisted store-wait
                # cannot stall load issue on the SP ring
                nc.scalar.dma_start(
                    out=out[vt * 128 : (vt + 1) * 128, bb * BB : (bb + 1) * BB],
                    in_=acc[:, :],
                )
    _split_multi_waits(nc)
    return nc


def _get_nc():
    global _nc_cache
    if _nc_cache is None:
        _nc_cache = _build_nc()
    return _nc_cache


def kernel(**inputs) -> np.ndarray:
    global LAST_EXEC_TIME_NS
    x = np.asarray(inputs["x"], dtype=np.float32)
    weight = np.asarray(inputs["weight"], dtype=np.float32)
    assert x.shape == (BATCH, N_VARS * P)
    assert weight.shape == (N_VARS, N_VARS * P)

    # wd[i, lag] = weight[i, lag*N_VARS + i]  (diagonal gather, no arithmetic)
    wd = np.einsum("ili->il", weight.reshape(N_VARS, P, N_VARS))

    # xT[j, b] = x[b, j]; j = lag*N_VARS + core*NV + v
    xT = np.ascontiguousarray(x.T)
    xTr = xT.reshape(P, N_CORES, NV, BATCH)

    in_maps = []
    for c in range(N_CORES):
        xt_c = np.ascontiguousarray(xTr[:, c]).reshape(P * NV, BATCH)
        wd_c = wd[c * NV : (c + 1) * NV]  # (NV, P)
        wds_c = np.ascontiguousarray(
            wd_c.reshape(VT, 128, P).transpose(1, 0, 2).reshape(128, VT * P)
        )
        in_maps.append({"xt": xt_c, "wds": wds_c})

    nc = _get_nc()
    trace = bool(int(os.environ.get("KERNEL_TRACE", "0")))
    try:
        res = run_bass_kernel_spmd(
            nc, in_maps, core_ids=list(range(N_CORES)), trace=trace
        )
    except ModuleNotFoundError:
        # axon containers without the NTFF profile hook can't trace
        # (BASS_TRACE env still forces trace inside run_bass_kernel_spmd)
        os.environ["BASS_NEVER_TRACE"] = "1"
        res = run_bass_kernel_spmd(
            nc, in_maps, core_ids=list(range(N_CORES)), trace=False
        )
    LAST_EXEC_TIME_NS = res.exec_time_ns

    out_full = np.empty((BATCH, N_VARS), dtype=np.float32)
    for c in range(N_CORES):
        out_full[:, c * NV : (c + 1) * NV] = np.asarray(res.results[c]["out_t"]).T
    return out_full


# revision 11
# speedup vs baseline: 1.0053x; 1.0053x over previous
"""DiagonalLinear kernel for 8x TRN2 NeuronCores (Bass/Tile).

Math: out[b, i] = sum_j x[b, j] * (weight * mask)[i, j] where
mask[i, lag*N_VARS + i] = 1. So the dense matmul collapses to

    out[b, i] = sum_{lag<P} x[b, lag*N_VARS + i] * wd[i, lag]
    wd[i, lag] = weight[i, lag*N_VARS + i]

i.e. an elementwise multiply-accumulate over P=8 lags — memory-bound on
streaming x (256 MB fp32) once, not a matmul.

Sharding: each of the 8 cores owns a contiguous slice of NV=256 variables
(fully independent given the diagonal mask). Per-core device layout puts
variables on SBUF partitions and batch on the free dim, so the per-lag
multiply needs only a per-partition scalar: lag 0 is a tensor_scalar_mul
(2x fp32 mode) and lags 1..7 are fused scalar_tensor_tensor
(acc = x*wd + acc), all on VectorE (~68 us busy). DMA is the bottleneck:
~36 MB per core at the ~360 GB/s HBM-per-core limit (~105 us). Loads are
issued per lag-pair (1 MB each) so compute streams behind the loads;
the last chunk loads per-lag so the final accumulation chain drains with
its loads. Cost-model (TimelineSim) predicted time: ~111 us/core.

Host side: extract the weight diagonal (pure gather), transpose x so each
core's shard is (P*NV, BATCH) contiguous, gather per-core outputs (NV,
BATCH) and transpose back to (BATCH, N_VARS).
"""

import os

import numpy as np

import concourse.bass as bass
import concourse.mybir as mybir
from concourse.bass_utils import run_bass_kernel_spmd
from concourse.tile import TileContext

N_VARS = 2048
P = 8
BATCH = 4096
N_CORES = 8
NV = N_VARS // N_CORES  # 256 variables per core
VT = NV // 128  # 2 partition tiles per core
BB = 1024  # batch tile width (free dim)
NB = BATCH // BB
LAG_GROUP = 2  # lags per load DMA (2 -> 1 MB transfers)

_nc_cache = None
LAST_EXEC_TIME_NS = None


def _split_multi_waits(nc):
    """Walrus in this toolchain accepts at most one sync-wait per
    instruction; hoist extras onto same-engine NoOps placed just before.
    Order-preserving and conservative: the engine stalls at the NoOp on the
    same condition it would have waited on at the instruction itself."""
    for fn in nc.m.functions:
        for blk in fn.blocks:
            out = []
            for ins in blk.instructions:
                si = ins.sync_info
                if si is not None and si.on_wait is not None and len(si.on_wait) > 1:
                    waits = list(si.on_wait)
                    for k, w in enumerate(waits[:-1]):
                        out.append(
                            mybir.InstNoOp(
                                name=f"{ins.name}_hw{k}",
                                engine=ins.engine,
                                ins=[],
                                outs=[],
                                sync_info=mybir.SyncInfo(on_wait=[w], on_update=[]),
                            )
                        )
                    ins.sync_info = mybir.SyncInfo(
                        on_wait=[waits[-1]], on_update=si.on_update
                    )
                out.append(ins)
            blk.instructions[:] = out


def _build_nc():
    nc = bass.Bass()
    xt = nc.dram_tensor("xt", [P * NV, BATCH], mybir.dt.float32, kind="ExternalInput")
    wds = nc.dram_tensor("wds", [128, VT * P], mybir.dt.float32, kind="ExternalInput")
    out = nc.dram_tensor("out_t", [NV, BATCH], mybir.dt.float32, kind="ExternalOutput")
    # view rows as (lag, v): row = lag*NV + v  ->  [v, lag, b]
    xt_v = xt.rearrange("(l v) b -> v l b", l=P)

    n_chunks = VT * NB
    with TileContext(nc) as tc:
        with (
            tc.tile_pool(name="w", bufs=1) as wpool,
            tc.tile_pool(name="x", bufs=3) as xpool,
            tc.tile_pool(name="acc", bufs=2) as apool,
        ):
            wtile = wpool.tile([128, VT * P], mybir.dt.float32)
            # ACT ring: keeps the SP ring free so the first x load issues
            # immediately
            nc.scalar.dma_start(out=wtile[:, :], in_=wds[:, :])
            for ci, (vt, bb) in enumerate(
                (vt, bb) for vt in range(VT) for bb in range(NB)
            ):
                # the last chunk loads per-lag so its accumulation chain
                # streams with the loads instead of waiting for all 8 lags
                lg = 1 if ci == n_chunks - 1 else LAG_GROUP
                t = xpool.tile([128, P, BB], mybir.dt.float32, tag="xload")
                for l0 in range(0, P, lg):
                    nc.sync.dma_start(
                        out=t[:, l0 : l0 + lg, :],
                        in_=xt_v[
                            vt * 128 : (vt + 1) * 128,
                            l0 : l0 + lg,
                            bb * BB : (bb + 1) * BB,
                        ],
                    )
                acc = apool.tile([128, BB], mybir.dt.float32, tag="acc")
                # acc = wd[:, lag0] * x_lag0  (per-partition scalar, 2x fp32)
                nc.vector.tensor_scalar_mul(
                    out=acc[:, :],
                    in0=t[:, 0, :],
                    scalar1=wtile[:, vt * P : vt * P + 1],
                )
                for lag in range(1, P - 1):
                    # acc = (x_lag * wd[:, lag]) + acc  (fused on VectorE)
                    nc.vector.scalar_tensor_tensor(
                        out=acc[:, :],
                        in0=t[:, lag, :],
                        scalar=wtile[:, vt * P + lag : vt * P + lag + 1],
                        in1=acc[:, :],
                        op0=mybir.AluOpType.mult,
                        op1=mybir.AluOpType.add,
                    )
                # final lag: on the last chunk, split the closing STT and
                # store into b-halves so the first half's store overlaps the
                # second half's accumulate — shortens the kernel tail
                lag = P - 1
                wl = wtile[:, vt * P + lag : vt * P + lag + 1]
                nsp = 2 if ci == n_chunks - 1 else 1
                S = BB // nsp
                for s in range(nsp):
                    nc.vector.scalar_tensor_tensor(
                        out=acc[:, s * S : (s + 1) * S],
                        in0=t[:, lag, s * S : (s + 1) * S],
                        scalar=wl,
                        in1=acc[:, s * S : (s + 1) * S],
                        op0=mybir.AluOpType.mult,
                        op1=mybir.AluOpType.add,
                    )
                    # store on the ACT HWDGE ring so a hoisted store-wait
                    # cannot stall load issue on the SP ring
                    nc.scalar.dma_start(
                        out=out[
                            vt * 128 : (vt + 1) * 128,
                            bb * BB + s * S : bb * BB + (s + 1) * S,
                        ],
                        in_=acc[:, s * S : (s + 1) * S],
                    )
    _split_multi_waits(nc)
    return nc


def _get_nc():
    global _nc_cache
    if _nc_cache is None:
        _nc_cache = _build_nc()
    return _nc_cache


def kernel(**inputs) -> np.ndarray:
    global LAST_EXEC_TIME_NS
    x = np.asarray(inputs["x"], dtype=np.float32)
    weight = np.asarray(inputs["weight"], dtype=np.float32)
    assert x.shape == (BATCH, N_VARS * P)
    assert weight.shape == (N_VARS, N_VARS * P)

    # wd[i, lag] = weight[i, lag*N_VARS + i]  (diagonal gather, no arithmetic)
    wd = np.einsum("ili->il", weight.reshape(N_VARS, P, N_VARS))

    # xT[j, b] = x[b, j]; j = lag*N_VARS + core*NV + v
    xT = np.ascontiguousarray(x.T)
    xTr = xT.reshape(P, N_CORES, NV, BATCH)

    in_maps = []
    for c in range(N_CORES):
        xt_c = np.ascontiguousarray(xTr[:, c]).reshape(P * NV, BATCH)
        wd_c = wd[c * NV : (c + 1) * NV]  # (NV, P)
        wds_c = np.ascontiguousarray(
            wd_c.reshape(VT, 128, P).transpose(1, 0, 2).reshape(128, VT * P)
        )
        in_maps.append({"xt": xt_c, "wds": wds_c})

    nc = _get_nc()
    trace = bool(int(os.environ.get("KERNEL_TRACE", "0")))
    try:
        res = run_bass_kernel_spmd(
            nc, in_maps, core_ids=list(range(N_CORES)), trace=trace
        )
    except ModuleNotFoundError:
        # axon containers without the NTFF profile hook can't trace
        # (BASS_TRACE env still forces trace inside run_bass_kernel_spmd)
        os.environ["BASS_NEVER_TRACE"] = "1"
        res = run_bass_kernel_spmd(
            nc, in_maps, core_ids=list(range(N_CORES)), trace=False
        )
    LAST_EXEC_TIME_NS = res.exec_time_ns

    out_full = np.empty((BATCH, N_VARS), dtype=np.float32)
    for c in range(N_CORES):
        out_full[:, c * NV : (c + 1) * NV] = np.asarray(res.results[c]["out_t"]).T
    return out_full
